# revision 7
# baseline (speedup 1.0000x reference)
"""AttentionWithPairBias distributed Trainium2 kernel (8 NeuronCores).

Sequence-parallel sharding: core c owns query rows i in [128c, 128(c+1)).
Per core: z shard [128, 1024, 128] (64MB f32 -> the memory roofline),
s replicated, all weights replicated. No collectives needed.

Pipeline per core:
  preamble: rmsnorm(s) (w_s folded into Wq/Wk/Wv/Wg), q^T/k^T (f32),
            v (bf16), g, via PE matmuls.
  phase 1 (z stream): SWDGE DMA casts z f32->bf16 into SBUF; HWDGE xbar
            DMA-transpose makes z^T tiles; DVE fused square+accum gives
            per-(i,j) sum(z^2); PE matmul (z^T stationary, Wz' moving)
            gives raw pair bias [j,12] per (i, jt); DVE scales by
            rsqrt(mean+eps) into B_stage[jt][j, i, h].
  phase 2 (attention, per head): scores psum = q^T k (PE) + bias via
            accumulating PE transposes of B_stage slices; ScalarE exp ->
            bf16; DVE multiplies by 0/1 mask, row-sums, normalizes;
            PE transposes attn; PE attn @ v accumulation.
  phase 3: o = (attn_out @ Wo + bo) * g -> DMA out.
"""

import os
from contextlib import ExitStack

import numpy as np

import concourse.bass as bass
import concourse.bacc as bacc
import concourse.tile as tile
import concourse.mybir as mybir
from concourse.masks import make_identity

S = 1024
CS = 384
CZ = 128
D = 32
H = 12
NCORES = 8
RB = S // NCORES  # 128 query rows per core
JT = S // 128     # 8 column tiles
CKS = CS // 128   # 3 contraction chunks of s-dim
EPS = 1e-5
INVD = 1.0 / np.sqrt(D)

F32 = mybir.dt.float32
BF16 = mybir.dt.bfloat16
I32 = mybir.dt.int32
AF = mybir.ActivationFunctionType
OP = mybir.AluOpType

IB = 32  # i-batch for bias psum banks (32*12*4B = 1536B <= bank)

# fraction of the square+accum (ms) tiles to run on ScalarE instead of DVE
MS_SCALAR_EVERY = 4  # every 4th i goes to ScalarE (tune from trace)


def _mm(nc, out, lhsT, rhs, start, stop, **kw):
    nc.tensor.matmul(out, lhsT, rhs, start=start, stop=stop, **kw)


def build(nc):
    s_full = nc.dram_tensor("s", [S, CS], F32, kind="ExternalInput").ap()
    s_loc = nc.dram_tensor("s_loc", [RB, CS], F32, kind="ExternalInput").ap()
    z_d = nc.dram_tensor("z", [RB, S, CZ], F32, kind="ExternalInput").ap()
    zm_d = nc.dram_tensor("z_mask", [RB, S], I32, kind="ExternalInput").ap()
    ws_d = nc.dram_tensor("w_s", [CS], F32, kind="ExternalInput").ap()
    wz_d = nc.dram_tensor("w_z", [CZ], F32, kind="ExternalInput").ap()
    Wz_d = nc.dram_tensor("Wz", [CZ, H], F32, kind="ExternalInput").ap()
    Wq_d = nc.dram_tensor("Wq", [CS, CS], F32, kind="ExternalInput").ap()
    Wk_d = nc.dram_tensor("Wk", [CS, CS], F32, kind="ExternalInput").ap()
    Wv_d = nc.dram_tensor("Wv", [CS, CS], F32, kind="ExternalInput").ap()
    Wg_d = nc.dram_tensor("Wg", [CS, CS], F32, kind="ExternalInput").ap()
    bg_d = nc.dram_tensor("bg", [CS], F32, kind="ExternalInput").ap()
    Wo_d = nc.dram_tensor("Wo", [CS, CS], F32, kind="ExternalInput").ap()
    bo_d = nc.dram_tensor("bo", [CS], F32, kind="ExternalInput").ap()
    out_d = nc.dram_tensor("out", [RB, CS], F32, kind="ExternalOutput").ap()

    with tile.TileContext(nc) as tc, ExitStack() as ctx:
        sg = ctx.enter_context(tc.tile_pool(name="singles", bufs=1))

        # ---------- constants / weights ----------
        ident_f = sg.tile([128, 128], F32)
        make_identity(nc, ident_f)
        ident_b = sg.tile([128, 128], BF16)
        make_identity(nc, ident_b)
        ones1 = sg.tile([1, 128], F32)
        nc.vector.memset(ones1, 1.0)
        eps_t = sg.tile([128, 1], F32)
        nc.vector.memset(eps_t, EPS)

        w_sb = {}
        for name, dram in (("Wq", Wq_d), ("Wk", Wk_d), ("Wv", Wv_d),
                           ("Wg", Wg_d), ("Wo", Wo_d)):
            t = sg.tile([128, CKS, CS], F32, tag=f"w_{name}")
            nc.sync.dma_start(out=t, in_=dram.rearrange("(k p) c -> p k c", p=128))
            w_sb[name] = t
        Wz_sb = sg.tile([128, H], F32)
        nc.sync.dma_start(out=Wz_sb, in_=Wz_d)
        ws_sb = sg.tile([128, CKS], F32)
        nc.sync.dma_start(out=ws_sb, in_=ws_d.rearrange("(k p) -> p k", p=128))
        wzv_sb = sg.tile([128, 1], F32)
        nc.sync.dma_start(out=wzv_sb, in_=wz_d.rearrange("(p o) -> p o", o=1))
        bg_sb = sg.tile([1, CS], F32)
        nc.sync.dma_start(out=bg_sb, in_=bg_d.rearrange("(o c) -> o c", o=1))
        bo_sb = sg.tile([1, CS], F32)
        nc.sync.dma_start(out=bo_sb, in_=bo_d.rearrange("(o c) -> o c", o=1))

        # fold w_s into Wq/Wk/Wv/Wg rows, w_z into Wz rows
        for name in ("Wq", "Wk", "Wv", "Wg"):
            for k in range(CKS):
                nc.vector.tensor_scalar_mul(
                    w_sb[name][:, k, :], w_sb[name][:, k, :], ws_sb[:, k:k + 1])
        nc.vector.tensor_scalar_mul(Wz_sb, Wz_sb, wzv_sb)
        Wz_bf = sg.tile([128, H], BF16)
        nc.vector.tensor_copy(out=Wz_bf, in_=Wz_sb)

        # mask -> bf16 0/1
        mask_bf = sg.tile([128, S], BF16)
        with tc.tile_pool(name="mtmp", bufs=1) as mp:
            mi = mp.tile([128, S], I32)
            nc.sync.dma_start(out=mi, in_=zm_d)
            nc.vector.tensor_copy(out=mask_bf, in_=mi)

        # ---------- rmsnorm(s) ----------
        s_r = sg.tile([128, JT, CS], F32)       # all rows, normalized (no w_s)
        nc.sync.dma_start(out=s_r, in_=s_full.rearrange("(t p) c -> p t c", p=128))
        s_rl = sg.tile([128, CS], F32)          # local rows, normalized
        nc.sync.dma_start(out=s_rl, in_=s_loc)

        with tc.tile_pool(name="pre_tmp", bufs=3) as pt:
            def norm_rows(ap):
                sq = pt.tile([128, CS], BF16, tag="sq")
                msum = pt.tile([128, 1], F32, tag="msum")
                nc.scalar.activation(out=sq, in_=ap, func=AF.Square,
                                     scale=float(1.0 / np.sqrt(CS)),
                                     accum_out=msum)
                nc.scalar.activation(out=msum, in_=msum, func=AF.Sqrt,
                                     bias=eps_t, scale=1.0)
                nc.vector.reciprocal(out=msum, in_=msum)
                nc.vector.tensor_scalar_mul(ap, ap, msum)

            for t in range(JT):
                norm_rows(s_r[:, t, :])
            norm_rows(s_rl)

        # ---------- transposes of s_r ----------
        s_rT = sg.tile([128, CKS, S], F32)    # [c, k, i]
        s_rTl = sg.tile([128, CKS, 128], F32)  # [c, k, local i]
        with tc.tile_pool(name="pre_ps", bufs=3, space="PSUM") as pp:
            for t in range(JT):
                for k in range(CKS):
                    ps = pp.tile([128, 128], F32, tag="tp")
                    _mm(nc, ps, s_r[:, t, bass.ts(k, 128)], ident_f, True, True,
                        is_transpose=True)
                    nc.scalar.copy(out=s_rT[:, k, bass.ts(t, 128)], in_=ps)
            for k in range(CKS):
                ps = pp.tile([128, 128], F32, tag="tp")
                _mm(nc, ps, s_rl[:, bass.ts(k, 128)], ident_f, True, True,
                    is_transpose=True)
                nc.scalar.copy(out=s_rTl[:, k, :], in_=ps)

            # ---------- qT (local), kT (full), v (bf16), g ----------
            qT = sg.tile([128, CKS, 128], F32)   # [hd_in_chunk, chunk, i_loc]
            kT = sg.tile([128, CKS, S], F32)     # [hd_in_chunk, chunk, j]
            v_sb = sg.tile([128, JT, CS], BF16)  # [j_in_tile, jt, hd]
            g_sb = sg.tile([128, CS], F32)

            for k in range(CKS):
                ps = pp.tile([128, 128], F32, tag="tp")
                for ck in range(CKS):
                    _mm(nc, ps, w_sb["Wq"][:, ck, bass.ts(k, 128)],
                        s_rTl[:, ck, :], ck == 0, ck == CKS - 1)
                nc.scalar.mul(out=qT[:, k, :], in_=ps, mul=float(INVD))
                for half in range(2):
                    ps2 = pp.tile([128, 512], F32, tag="big")
                    for ck in range(CKS):
                        _mm(nc, ps2, w_sb["Wk"][:, ck, bass.ts(k, 128)],
                            s_rT[:, ck, bass.ts(half, 512)], ck == 0, ck == CKS - 1)
                    nc.scalar.copy(out=kT[:, k, bass.ts(half, 512)], in_=ps2)
            for jc in range(JT):
                ps2 = pp.tile([128, 512], F32, tag="big")
                for ck in range(CKS):
                    _mm(nc, ps2[:, 0:CS], s_rT[:, ck, bass.ts(jc, 128)],
                        w_sb["Wv"][:, ck, :], ck == 0, ck == CKS - 1)
                nc.scalar.copy(out=v_sb[:, jc, :], in_=ps2[:, 0:CS])
            ps2 = pp.tile([128, 512], F32, tag="big")
            for ck in range(CKS):
                _mm(nc, ps2[:, 0:CS], s_rTl[:, ck, :], w_sb["Wg"][:, ck, :],
                    ck == 0, False)
            _mm(nc, ps2[:, 0:CS], ones1, bg_sb, False, True)
            nc.scalar.copy(out=g_sb, in_=ps2[:, 0:CS])

        # ---------- phase 1: z stream ----------
        ms_st = sg.tile([128, JT, RB], F32)           # [j, jt, i]
        B_st = sg.tile([128, JT, RB, H], F32)         # [j, jt, i, h]

        with tc.tile_pool(name="znat", bufs=3) as znp, \
             tc.tile_pool(name="znT", bufs=3) as ztp, \
             tc.tile_pool(name="sscr", bufs=2) as scrp, \
             tc.tile_pool(name="bias_ps", bufs=1, space="PSUM") as bpp:
            bias_ps = {}
            for i in range(RB):
                zn = znp.tile([128, JT, CZ], BF16, tag="zn")
                nc.gpsimd.dma_start(
                    out=zn, in_=z_d[i].rearrange("(jt j) c -> j jt c", j=128))
                zt = ztp.tile([128, JT, 128], BF16, tag="zt")  # [c, jt, j]
                nc.sync.dma_start(out=zt, in_=zn, transpose=True)

                ib = i % IB
                if ib == 0:
                    for jt in range(JT):
                        bias_ps[jt] = bpp.tile([128, IB, H], F32, tag=f"b{jt}",
                                           name=f"bias_ps{jt}")
                for jt in range(JT):
                    scr = scrp.tile([128, CZ], BF16, tag="scr")
                    if i % MS_SCALAR_EVERY == 0:
                        nc.scalar.activation(
                            out=scr, in_=zn[:, jt, :], func=AF.Square,
                            scale=float(1.0 / np.sqrt(CZ)),
                            accum_out=ms_st[:, jt, i:i + 1])
                    else:
                        nc.vector.scalar_tensor_tensor(
                            out=scr, in0=zn[:, jt, :], scalar=float(1.0 / CZ),
                            in1=zn[:, jt, :], op0=OP.mult, op1=OP.mult,
                            accum_out=ms_st[:, jt, i:i + 1])
                    _mm(nc, bias_ps[jt][:, ib, :], zt[:, jt, :], Wz_bf,
                        ib == 0, ib == IB - 1)

                if ib == IB - 1:
                    i0 = i - (IB - 1)
                    for jt in range(JT):
                        # rs = 1/sqrt(ms + eps), in place in ms_st
                        nc.scalar.activation(
                            out=ms_st[:, jt, i0:i + 1], in_=ms_st[:, jt, i0:i + 1],
                            func=AF.Sqrt, bias=eps_t, scale=1.0)
                        nc.vector.reciprocal(out=ms_st[:, jt, i0:i + 1],
                                             in_=ms_st[:, jt, i0:i + 1])
                        rs_b = bass.AP(
                            tensor=ms_st.tensor,
                            offset=ms_st.offset + (jt * RB + i0),
                            ap=[ms_st.ap[0], [1, IB], [0, H]])
                        nc.vector.tensor_tensor(
                            out=B_st[:, jt, i0:i0 + IB, :],
                            in0=bias_ps[jt], in1=rs_b, op=OP.mult)

        # ---------- phase 2: attention ----------
        with tc.tile_pool(name="sc_ps", bufs=2, space="PSUM") as scp, \
             tc.tile_pool(name="at_ps", bufs=2, space="PSUM") as atp, \
             tc.tile_pool(name="o_ps", bufs=2, space="PSUM") as opp, \
             tc.tile_pool(name="att_sb", bufs=3) as asb, \
             tc.tile_pool(name="attT_sb", bufs=3) as atsb, \
             tc.tile_pool(name="den_sb", bufs=2) as dsb:
            oT_sb = sg.tile([128, CKS, 128], F32)   # [hd_in_chunk, chunk, i]
            for h in range(H):
                ck, hp = divmod(h, 4)
                sc = scp.tile([128, S], F32, tag="sc")
                for half in range(2):
                    _mm(nc, sc[:, bass.ts(half, 512)],
                        qT[bass.ts(hp, 32), ck, :],
                        kT[bass.ts(hp, 32), ck, bass.ts(half, 512)],
                        True, False, tile_position=(32 * hp, 0))
                for jt in range(JT):
                    b_slice = bass.AP(
                        tensor=B_st.tensor,
                        offset=B_st.offset + (jt * RB * H + h),
                        ap=[B_st.ap[0], [H, RB]])
                    _mm(nc, sc[:, bass.ts(jt, 128)], b_slice, ident_f,
                        False, jt in (3, 7), is_transpose=True)
                att = asb.tile([128, S], BF16, tag="att")
                nc.scalar.activation(out=att, in_=sc, func=AF.Exp)
                nc.vector.tensor_tensor(out=att, in0=att, in1=mask_bf, op=OP.mult)
                den = dsb.tile([128, 1], F32, tag="den")
                nc.vector.tensor_reduce(out=den, in_=att, axis=mybir.AxisListType.X,
                                        op=OP.add)
                nc.vector.reciprocal(out=den, in_=den)
                nc.vector.tensor_scalar_mul(att, att, den)
                o_ps = opp.tile([32, 128], F32, tag="o")
                for jc in range(JT):
                    at_ps = atp.tile([128, 128], BF16, tag="atT")
                    _mm(nc, at_ps, att[:, bass.ts(jc, 128)], ident_b, True, True,
                        is_transpose=True)
                    atT = atsb.tile([128, 128], BF16, tag="atTs")
                    nc.scalar.copy(out=atT, in_=at_ps)
                    _mm(nc, o_ps, v_sb[:, jc, bass.ts(h, 32)], atT,
                        jc == 0, jc == JT - 1)
                nc.scalar.copy(out=oT_sb[bass.ts(hp, 32), ck, :], in_=o_ps)

            # ---------- phase 3: output ----------
            fin = scp.tile([128, S], F32, tag="sc")
            for k in range(CKS):
                _mm(nc, fin[:, 0:CS], oT_sb[:, k, :], w_sb["Wo"][:, k, :],
                    k == 0, False)
            _mm(nc, fin[:, 0:CS], ones1, bo_sb, False, True)
            out_sb = sg.tile([128, CS], F32)
            nc.vector.tensor_tensor(out=out_sb, in0=fin[:, 0:CS], in1=g_sb,
                                    op=OP.mult)
            nc.sync.dma_start(out=out_d, in_=out_sb)

    nc.compile()
    return nc


_NC_CACHE = None


def _get_nc():
    global _NC_CACHE
    if _NC_CACHE is None:
        nc = bacc.Bacc("TRN2", target_bir_lowering=False, debug=False,
                       enable_asserts=False)
        _NC_CACHE = build(nc)
    return _NC_CACHE


def make_in_maps(s, z, z_mask, w_s, w_z, Wz, Wq, Wk, Wv, Wg, bg, Wo, bo):
    f = lambda a: np.ascontiguousarray(np.asarray(a), dtype=np.float32)
    s = f(s)
    shared = dict(s=s, w_s=f(w_s), w_z=f(w_z), Wz=f(Wz), Wq=f(Wq), Wk=f(Wk),
                  Wv=f(Wv), Wg=f(Wg), bg=f(bg), Wo=f(Wo), bo=f(bo))
    zmask = np.ascontiguousarray(np.asarray(z_mask), dtype=np.int32)
    z = f(z)
    in_maps = []
    for c in range(NCORES):
        r0, r1 = c * RB, (c + 1) * RB
        m = dict(shared)
        m["s_loc"] = np.ascontiguousarray(s[r0:r1])
        m["z"] = np.ascontiguousarray(z[r0:r1])
        m["z_mask"] = np.ascontiguousarray(zmask[r0:r1])
        in_maps.append(m)
    return in_maps


def kernel(**inputs):
    from concourse import bass_utils
    nc = _get_nc()
    in_maps = make_in_maps(**inputs)
    res = bass_utils.run_bass_kernel_spmd(nc, in_maps, core_ids=list(range(NCORES)))
    out = np.concatenate([res.results[c]["out"] for c in range(NCORES)], axis=0)
    return out.astype(np.float32)


# revision 9
# speedup vs baseline: 1.6959x; 1.6959x over previous
"""AttentionWithPairBias distributed Trainium2 kernel (8 NeuronCores).

Sequence-parallel sharding: core c owns query rows i in [128c, 128(c+1)).
Per core: z shard [128, 1024, 128] (64MB f32 -> the memory roofline),
s replicated, all weights replicated. No collectives needed.

Pipeline per core:
  preamble: rmsnorm(s) (w_s folded into Wq/Wk/Wv/Wg), q^T/k^T (f32),
            v (bf16), g, via PE matmuls.
  phase 1 (z stream): SWDGE DMA casts z f32->bf16 into SBUF; HWDGE xbar
            DMA-transpose makes z^T tiles; DVE fused square+accum gives
            per-(i,j) sum(z^2); PE matmul (z^T stationary, Wz' moving)
            gives raw pair bias [j,12] per (i, jt); DVE scales by
            rsqrt(mean+eps) into B_stage[jt][j, i, h].
  phase 2 (attention, per head): scores psum = q^T k (PE) + bias via
            accumulating PE transposes of B_stage slices; ScalarE exp ->
            bf16; DVE multiplies by 0/1 mask, row-sums, normalizes;
            PE transposes attn; PE attn @ v accumulation.
  phase 3: o = (attn_out @ Wo + bo) * g -> DMA out.
"""

import os
from contextlib import ExitStack

import numpy as np

import concourse.bass as bass
import concourse.bacc as bacc
import concourse.tile as tile
import concourse.mybir as mybir
from concourse.masks import make_identity

S = 1024
CS = 384
CZ = 128
D = 32
H = 12
NCORES = 8
RB = S // NCORES  # 128 query rows per core
JT = S // 128     # 8 column tiles
CKS = CS // 128   # 3 contraction chunks of s-dim
EPS = 1e-5
INVD = 1.0 / np.sqrt(D)

F32 = mybir.dt.float32
BF16 = mybir.dt.bfloat16
I32 = mybir.dt.int32
AF = mybir.ActivationFunctionType
OP = mybir.AluOpType

IB = 32  # i-batch for bias psum banks (32*12*4B = 1536B <= bank)

# fraction of the square+accum (ms) tiles to run on ScalarE instead of DVE
MS_SCALAR_EVERY = 4  # every 4th i goes to ScalarE (tune from trace)


def _mm(nc, out, lhsT, rhs, start, stop, **kw):
    nc.tensor.matmul(out, lhsT, rhs, start=start, stop=stop, **kw)


def build(nc):
    s_full = nc.dram_tensor("s", [S, CS], F32, kind="ExternalInput").ap()
    s_loc = nc.dram_tensor("s_loc", [RB, CS], F32, kind="ExternalInput").ap()
    z_d = nc.dram_tensor("z", [RB, S, CZ], F32, kind="ExternalInput").ap()
    zm_d = nc.dram_tensor("z_mask", [RB, S], I32, kind="ExternalInput").ap()
    ws_d = nc.dram_tensor("w_s", [CS], F32, kind="ExternalInput").ap()
    wz_d = nc.dram_tensor("w_z", [CZ], F32, kind="ExternalInput").ap()
    Wz_d = nc.dram_tensor("Wz", [CZ, H], F32, kind="ExternalInput").ap()
    Wq_d = nc.dram_tensor("Wq", [CS, CS], F32, kind="ExternalInput").ap()
    Wk_d = nc.dram_tensor("Wk", [CS, CS], F32, kind="ExternalInput").ap()
    Wv_d = nc.dram_tensor("Wv", [CS, CS], F32, kind="ExternalInput").ap()
    Wg_d = nc.dram_tensor("Wg", [CS, CS], F32, kind="ExternalInput").ap()
    bg_d = nc.dram_tensor("bg", [CS], F32, kind="ExternalInput").ap()
    Wo_d = nc.dram_tensor("Wo", [CS, CS], F32, kind="ExternalInput").ap()
    bo_d = nc.dram_tensor("bo", [CS], F32, kind="ExternalInput").ap()
    out_d = nc.dram_tensor("out", [RB, CS], F32, kind="ExternalOutput").ap()

    with tile.TileContext(nc) as tc, ExitStack() as ctx:
        sg = ctx.enter_context(tc.tile_pool(name="singles", bufs=1))

        # ---------- constants / weights ----------
        ident_f = sg.tile([128, 128], F32)
        make_identity(nc, ident_f)
        ident_b = sg.tile([128, 128], BF16)
        make_identity(nc, ident_b)
        ones1 = sg.tile([1, 128], F32)
        nc.vector.memset(ones1, 1.0)
        eps_t = sg.tile([128, 1], F32)
        nc.vector.memset(eps_t, EPS)

        pre_sg = ctx.enter_context(tc.tile_pool(name="pre_sg", bufs=1))
        w_sb = {}
        for name, dram in (("Wq", Wq_d), ("Wk", Wk_d), ("Wv", Wv_d),
                           ("Wg", Wg_d), ("Wo", Wo_d)):
            pool = sg if name == "Wo" else pre_sg
            t = pool.tile([128, CKS, CS], F32, tag=f"w_{name}", name=f"w_{name}")
            nc.sync.dma_start(out=t, in_=dram.rearrange("(k p) c -> p k c", p=128))
            w_sb[name] = t
        Wz_sb = sg.tile([128, H], F32)
        nc.sync.dma_start(out=Wz_sb, in_=Wz_d)
        ws_sb = sg.tile([128, CKS], F32)
        nc.sync.dma_start(out=ws_sb, in_=ws_d.rearrange("(k p) -> p k", p=128))
        wzv_sb = sg.tile([128, 1], F32)
        nc.sync.dma_start(out=wzv_sb, in_=wz_d.rearrange("(p o) -> p o", o=1))
        bg_sb = sg.tile([1, CS], F32)
        nc.sync.dma_start(out=bg_sb, in_=bg_d.rearrange("(o c) -> o c", o=1))
        bo_sb = sg.tile([1, CS], F32)
        nc.sync.dma_start(out=bo_sb, in_=bo_d.rearrange("(o c) -> o c", o=1))

        # fold w_s into Wq/Wk/Wv/Wg rows, w_z into Wz rows
        for name in ("Wq", "Wk", "Wv", "Wg"):
            for k in range(CKS):
                nc.vector.tensor_scalar_mul(
                    w_sb[name][:, k, :], w_sb[name][:, k, :], ws_sb[:, k:k + 1])
        nc.vector.tensor_scalar_mul(Wz_sb, Wz_sb, wzv_sb)
        Wz_bf = sg.tile([128, H], BF16)
        nc.vector.tensor_copy(out=Wz_bf, in_=Wz_sb)

        # mask -> bf16 0/1
        mask_bf = sg.tile([128, S], BF16)
        with tc.tile_pool(name="mtmp", bufs=1) as mp:
            mi = mp.tile([128, S], I32)
            nc.sync.dma_start(out=mi, in_=zm_d)
            nc.vector.tensor_copy(out=mask_bf, in_=mi)

        # ---------- rmsnorm(s) ----------
        s_r = pre_sg.tile([128, JT, CS], F32)   # all rows, normalized (no w_s)
        nc.sync.dma_start(out=s_r, in_=s_full.rearrange("(t p) c -> p t c", p=128))
        s_rl = pre_sg.tile([128, CS], F32)      # local rows, normalized
        nc.sync.dma_start(out=s_rl, in_=s_loc)

        with tc.tile_pool(name="pre_tmp", bufs=3) as pt:
            def norm_rows(ap):
                sq = pt.tile([128, CS], BF16, tag="sq")
                msum = pt.tile([128, 1], F32, tag="msum")
                nc.scalar.activation(out=sq, in_=ap, func=AF.Square,
                                     scale=float(1.0 / np.sqrt(CS)),
                                     accum_out=msum)
                nc.scalar.activation(out=msum, in_=msum, func=AF.Sqrt,
                                     bias=eps_t, scale=1.0)
                nc.vector.reciprocal(out=msum, in_=msum)
                nc.vector.tensor_scalar_mul(ap, ap, msum)

            for t in range(JT):
                norm_rows(s_r[:, t, :])
            norm_rows(s_rl)

        # ---------- transposes of s_r ----------
        s_rT = sg.tile([128, CKS, S], F32)    # [c, k, i]
        s_rTl = sg.tile([128, CKS, 128], F32)  # [c, k, local i]
        with tc.tile_pool(name="pre_ps", bufs=3, space="PSUM") as pp:
            for t in range(JT):
                for k in range(CKS):
                    ps = pp.tile([128, 128], F32, tag="tp")
                    _mm(nc, ps, s_r[:, t, bass.ts(k, 128)], ident_f, True, True,
                        is_transpose=True)
                    nc.scalar.copy(out=s_rT[:, k, bass.ts(t, 128)], in_=ps)
            for k in range(CKS):
                ps = pp.tile([128, 128], F32, tag="tp")
                _mm(nc, ps, s_rl[:, bass.ts(k, 128)], ident_f, True, True,
                    is_transpose=True)
                nc.scalar.copy(out=s_rTl[:, k, :], in_=ps)

            # ---------- qT (local), kT (full), v (bf16), g ----------
            qT = sg.tile([128, CKS, 128], F32)   # [hd_in_chunk, chunk, i_loc]
            kT = sg.tile([128, CKS, S], F32)     # [hd_in_chunk, chunk, j]
            v_sb = sg.tile([128, JT, CS], BF16)  # [j_in_tile, jt, hd]
            g_sb = sg.tile([128, CS], F32)

            for k in range(CKS):
                ps = pp.tile([128, 128], F32, tag="tp")
                for ck in range(CKS):
                    _mm(nc, ps, w_sb["Wq"][:, ck, bass.ts(k, 128)],
                        s_rTl[:, ck, :], ck == 0, ck == CKS - 1)
                nc.scalar.mul(out=qT[:, k, :], in_=ps, mul=float(INVD))
                for half in range(2):
                    ps2 = pp.tile([128, 512], F32, tag="big")
                    for ck in range(CKS):
                        _mm(nc, ps2, w_sb["Wk"][:, ck, bass.ts(k, 128)],
                            s_rT[:, ck, bass.ts(half, 512)], ck == 0, ck == CKS - 1)
                    nc.scalar.copy(out=kT[:, k, bass.ts(half, 512)], in_=ps2)
            for jc in range(JT):
                ps2 = pp.tile([128, 512], F32, tag="big")
                for ck in range(CKS):
                    _mm(nc, ps2[:, 0:CS], s_rT[:, ck, bass.ts(jc, 128)],
                        w_sb["Wv"][:, ck, :], ck == 0, ck == CKS - 1)
                nc.scalar.copy(out=v_sb[:, jc, :], in_=ps2[:, 0:CS])
            ps2 = pp.tile([128, 512], F32, tag="big")
            for ck in range(CKS):
                _mm(nc, ps2[:, 0:CS], s_rTl[:, ck, :], w_sb["Wg"][:, ck, :],
                    ck == 0, False)
            _mm(nc, ps2[:, 0:CS], ones1, bg_sb, False, True)
            nc.scalar.copy(out=g_sb, in_=ps2[:, 0:CS])

        # ---------- phase 1: z stream ----------
        # batches of BI=4 query rows: one big cast DMA + one big xbar
        # transpose + one gpsimd square + one DVE 3D reduce per batch.
        BI = 4
        NB = RB // BI               # 32 batches
        ms_st = sg.tile([128, JT, RB], F32)           # [j, jt, i]
        B_st = sg.tile([128, JT, RB, H], F32)         # [j, jt, i, h]

        with tc.tile_pool(name="znat", bufs=3) as znp, \
             tc.tile_pool(name="znT", bufs=3) as ztp, \
             tc.tile_pool(name="sqp", bufs=2) as sqp, \
             tc.tile_pool(name="bias_ps", bufs=1, space="PSUM") as bpp:
            bias_ps = {}
            for b in range(NB):
                i0 = b * BI
                zn = znp.tile([128, BI, JT, CZ], BF16, tag="zn")
                nc.gpsimd.dma_start(
                    out=zn,
                    in_=z_d[i0:i0 + BI].rearrange("i (jt j) c -> j i jt c", j=128))
                zt = ztp.tile([128, BI * JT, 128], BF16, tag="zt")  # [c, n, j]
                nc.sync.dma_start(out=zt, in_=zn, transpose=True)

                # ms for the whole batch: square on gpsimd, 3D reduce on DVE
                sq = sqp.tile([128, BI, JT, CZ], BF16, tag="sq")
                nc.gpsimd.tensor_mul(sq, zn, zn)
                ms_out = bass.AP(
                    tensor=ms_st.tensor, offset=ms_st.offset + i0,
                    ap=[ms_st.ap[0], [1, BI], [RB, JT]])
                nc.vector.tensor_reduce(out=ms_out, in_=sq,
                                        axis=mybir.AxisListType.X, op=OP.add)

                if i0 % IB == 0:
                    for jt in range(JT):
                        bias_ps[jt] = bpp.tile([128, IB, H], F32, tag=f"b{jt}",
                                               name=f"bias_ps{jt}")
                for ii in range(BI):
                    ib = (i0 + ii) % IB
                    for jt in range(JT):
                        _mm(nc, bias_ps[jt][:, ib, :], zt[:, ii * JT + jt, :],
                            Wz_bf, ib == 0, ib == IB - 1)

                if (i0 + BI) % IB == 0:
                    g0 = i0 + BI - IB
                    for jt in range(JT):
                        # rs = 1/sqrt(ms/CZ + eps), in place in ms_st
                        nc.scalar.activation(
                            out=ms_st[:, jt, g0:g0 + IB],
                            in_=ms_st[:, jt, g0:g0 + IB],
                            func=AF.Sqrt, bias=eps_t, scale=float(1.0 / CZ))
                        nc.vector.reciprocal(out=ms_st[:, jt, g0:g0 + IB],
                                             in_=ms_st[:, jt, g0:g0 + IB])
                        rs_b = bass.AP(
                            tensor=ms_st.tensor,
                            offset=ms_st.offset + (jt * RB + g0),
                            ap=[ms_st.ap[0], [1, IB], [0, H]])
                        nc.vector.tensor_tensor(
                            out=B_st[:, jt, g0:g0 + IB, :],
                            in0=bias_ps[jt], in1=rs_b, op=OP.mult)

        # ---------- phase 2: attention ----------
        with tc.tile_pool(name="sc_ps", bufs=2, space="PSUM") as scp, \
             tc.tile_pool(name="at_ps", bufs=2, space="PSUM") as atp, \
             tc.tile_pool(name="o_ps", bufs=2, space="PSUM") as opp, \
             tc.tile_pool(name="att_sb", bufs=3) as asb, \
             tc.tile_pool(name="attT_sb", bufs=3) as atsb, \
             tc.tile_pool(name="den_sb", bufs=2) as dsb:
            oT_sb = sg.tile([128, CKS, 128], F32)   # [hd_in_chunk, chunk, i]
            for h in range(H):
                ck, hp = divmod(h, 4)
                sc = scp.tile([128, S], F32, tag="sc")
                for half in range(2):
                    _mm(nc, sc[:, bass.ts(half, 512)],
                        qT[bass.ts(hp, 32), ck, :],
                        kT[bass.ts(hp, 32), ck, bass.ts(half, 512)],
                        True, False, tile_position=(32 * hp, 0))
                for jt in range(JT):
                    b_slice = bass.AP(
                        tensor=B_st.tensor,
                        offset=B_st.offset + (jt * RB * H + h),
                        ap=[B_st.ap[0], [H, RB]])
                    _mm(nc, sc[:, bass.ts(jt, 128)], b_slice, ident_f,
                        False, jt in (3, 7), is_transpose=True)
                att = asb.tile([128, S], BF16, tag="att")
                nc.scalar.activation(out=att, in_=sc, func=AF.Exp)
                nc.vector.tensor_tensor(out=att, in0=att, in1=mask_bf, op=OP.mult)
                den = dsb.tile([128, 1], F32, tag="den")
                nc.vector.tensor_reduce(out=den, in_=att, axis=mybir.AxisListType.X,
                                        op=OP.add)
                nc.vector.reciprocal(out=den, in_=den)
                nc.vector.tensor_scalar_mul(att, att, den)
                o_ps = opp.tile([32, 128], F32, tag="o")
                for jc in range(JT):
                    at_ps = atp.tile([128, 128], BF16, tag="atT")
                    _mm(nc, at_ps, att[:, bass.ts(jc, 128)], ident_b, True, True,
                        is_transpose=True)
                    atT = atsb.tile([128, 128], BF16, tag="atTs")
                    nc.scalar.copy(out=atT, in_=at_ps)
                    _mm(nc, o_ps, v_sb[:, jc, bass.ts(h, 32)], atT,
                        jc == 0, jc == JT - 1)
                nc.scalar.copy(out=oT_sb[bass.ts(hp, 32), ck, :], in_=o_ps)

            # ---------- phase 3: output ----------
            fin = scp.tile([128, S], F32, tag="sc")
            for k in range(CKS):
                _mm(nc, fin[:, 0:CS], oT_sb[:, k, :], w_sb["Wo"][:, k, :],
                    k == 0, False)
            _mm(nc, fin[:, 0:CS], ones1, bo_sb, False, True)
            out_sb = sg.tile([128, CS], F32)
            nc.vector.tensor_tensor(out=out_sb, in0=fin[:, 0:CS], in1=g_sb,
                                    op=OP.mult)
            nc.sync.dma_start(out=out_d, in_=out_sb)

    nc.compile()
    return nc


_NC_CACHE = None


def _get_nc():
    global _NC_CACHE
    if _NC_CACHE is None:
        nc = bacc.Bacc("TRN2", target_bir_lowering=False, debug=False,
                       enable_asserts=False)
        _NC_CACHE = build(nc)
    return _NC_CACHE


def make_in_maps(s, z, z_mask, w_s, w_z, Wz, Wq, Wk, Wv, Wg, bg, Wo, bo):
    f = lambda a: np.ascontiguousarray(np.asarray(a), dtype=np.float32)
    s = f(s)
    shared = dict(s=s, w_s=f(w_s), w_z=f(w_z), Wz=f(Wz), Wq=f(Wq), Wk=f(Wk),
                  Wv=f(Wv), Wg=f(Wg), bg=f(bg), Wo=f(Wo), bo=f(bo))
    zmask = np.ascontiguousarray(np.asarray(z_mask), dtype=np.int32)
    z = f(z)
    in_maps = []
    for c in range(NCORES):
        r0, r1 = c * RB, (c + 1) * RB
        m = dict(shared)
        m["s_loc"] = np.ascontiguousarray(s[r0:r1])
        m["z"] = np.ascontiguousarray(z[r0:r1])
        m["z_mask"] = np.ascontiguousarray(zmask[r0:r1])
        in_maps.append(m)
    return in_maps


def kernel(**inputs):
    from concourse import bass_utils
    nc = _get_nc()
    in_maps = make_in_maps(**inputs)
    res = bass_utils.run_bass_kernel_spmd(nc, in_maps, core_ids=list(range(NCORES)))
    out = np.concatenate([res.results[c]["out"] for c in range(NCORES)], axis=0)
    return out.astype(np.float32)
